# revision 12
# baseline (speedup 1.0000x reference)
"""Trainium2 Bass kernel for nn_DecoderAttention3 (2-layer LSTM decoder with
attention + vocab projection), distributed over 8 NeuronCores.

Single fused module, data-parallel over batch (8 batches/core), with the two
LSTM layers SKEWED: layer 1 runs step t while layer 2 runs step t-D.  Between
them, attention + the lin projection are per-step pipeline stages:

  tick t:  whh(l0,t) -> chain(l0,t) -> scores(l0,t-1) -> soft+attn(l0,t-2)
           -> lin(l0,t-2) -> same for l1 at t-D

Recurrence is the transposed-gates formulation: gates [128 gate-part, 16
chunks, batch] accumulate in PSUM; W_hh chunks are stationary, the batch
(N=8) is the moving operand; the x@W_ih.T + bias contribution is batched 4
steps per matmul group (N=32) and is off the critical chain.  The cell
update runs on Act/DVE/Pool directly in [gate-part, batch] layout (no
transposes); h lands matmul-ready in catT.

Attention per step: scores via per-batch stationary enc chunks (N=1 matmuls),
exp in f32 (no max subtraction; |scores| <~ 55 empirically, same as the
reference's own headroom), softmax normalization folded into a rank-1
broadcast matmul, attention readout as 32 N=1 f32r matmuls.

Projection: x3T @ out_w.T over the full vocab per core (batch-sharded),
out_w streamed from DRAM in 1024-vocab chunks, fp16 logits to DRAM; host
adds out_b and casts to fp32.

Gate columns are host-permuted to [i, f, o, g] with the g block pre-scaled
by 2 so one sigmoid covers all gates (tanh(x) = 2*sigmoid(2x) - 1).
"""
import sys
for p in ('/opt/trn_rl_repo', '/root/.axon_site/_ro/trn_rl_repo'):
    if p not in sys.path:
        sys.path.insert(0, p)

import contextlib

import numpy as np

import concourse.bass as bass
import concourse.tile as tile
from concourse import bacc, mybir

F32 = mybir.dt.float32
F32R = mybir.dt.float32r
F16 = mybir.dt.float16
AF = mybir.ActivationFunctionType
OP = mybir.AluOpType

B, T, S, H, V, L = 64, 60, 60, 512, 32000, 2
NCORES = 8
BL = B // NCORES          # 8 local batches per core
Tp = 64                   # padded T
NT = T * BL               # 480
NTp = Tp * BL             # 512
G4 = 4 * H                # 2048
NGC = G4 // 128           # 16 gate chunks
NSB = 4                   # steps per xg matmul batch
D = 8                     # layer-2 skew (ticks)
VC = 1024                 # vocab chunk for streamed projection
NVC = V // VC             # 32
NEG = -30000.0

# arena column layout (f32 elements within a [128, 128] PSUM arena)
A_SC = 0      # scores [60, 8]
A_Z = 8       # Z      [1, 8]
A_RBC = 16    # rbc    [128, 8]
A_ATT = 24    # att    [128, 4*8]
A_X2 = 56     # x2     [128, 4*8]


def _build():
    nc = bacc.Bacc("TRN2", target_bir_lowering=False, debug=False, num_devices=NCORES)
    d = {}

    def inp(name, shape, dt=F16):
        d[name] = nc.dram_tensor(name, shape, dt, kind="ExternalInput").ap()

    inp("x1T", [H, NTp])
    inp("encT", [H, BL, S])
    inp("encs", [S, BL, H], F32R)
    inp("mask2", [BL, S])
    inp("wihT", [L, H, G4])
    inp("whhT", [L, H, G4])
    inp("bsum", [L, 1, G4])
    inp("linT", [L, 2 * H, H])
    inp("linb", [L, 1, H])
    inp("wT", [H, V])
    inp("ones", [1, H])
    inp("ident8", [8, 8])
    inp("ones60c", [S, 1], F32R)
    inp("onesr", [1, 128], F32R)
    d["out"] = nc.dram_tensor("out", [NT, V], F16, kind="ExternalOutput").ap()

    with tile.TileContext(nc) as tc:
        _body(nc, tc, d)
    nc.compile()
    return nc


def _body(nc, tc, d):
    ctx = contextlib.ExitStack()
    with ctx:
        ctx.enter_context(nc.allow_low_precision(
            reason="fp16 activations/weights intended; fp32 PSUM accumulation"))
        const = ctx.enter_context(tc.tile_pool(name="const", bufs=1))
        xpool = ctx.enter_context(tc.tile_pool(name="xpool", bufs=1))
        wpool = ctx.enter_context(tc.tile_pool(name="wpool", bufs=1))
        small = ctx.enter_context(tc.tile_pool(name="small", bufs=3))
        cpool = ctx.enter_context(tc.tile_pool(name="cpool", bufs=2))
        wstr = ctx.enter_context(tc.tile_pool(name="wstr", bufs=2))
        ostg = ctx.enter_context(tc.tile_pool(name="ostg", bufs=4))

        # ---- consts ----
        ident8 = const.tile([8, 8], F16)
        nc.sync.dma_start(ident8[:], d["ident8"][:])
        ones = const.tile([1, H], F16)
        nc.sync.dma_start(ones[:], d["ones"][:])
        ones60c = const.tile([S, 1], F32R)
        nc.sync.dma_start(ones60c[:], d["ones60c"][:])
        onesr = const.tile([1, 128], F32R)
        nc.sync.dma_start(onesr[:], d["onesr"][:])
        mask2_sb = const.tile([BL, S], F16)
        nc.sync.dma_start(mask2_sb[:], d["mask2"][:])
        bsum_sb = const.tile([1, L, G4], F16)
        for l in range(L):
            nc.sync.dma_start(bsum_sb[:, l, :], d["bsum"][l])
        linb_sb = const.tile([1, L, H], F16)
        for l in range(L):
            nc.sync.dma_start(linb_sb[:, l, :], d["linb"][l])

        # ---- big inputs ----
        x1T_sb = xpool.tile([128, 4, NTp], F16, tag="x1T")
        wih = [wpool.tile([128, 4, G4], F16, tag=f"wih{l}", name=f"wih{l}") for l in range(L)]
        whh = [wpool.tile([128, 4, G4], F16, tag=f"whh{l}", name=f"whh{l}") for l in range(L)]
        lin_sb = [wpool.tile([128, 8, H], F16, tag=f"lin{l}", name=f"lin{l}") for l in range(L)]
        for k in range(4):
            nc.sync.dma_start(x1T_sb[:, k, :], d["x1T"][k * 128:(k + 1) * 128, :])
        for k in range(4):
            nc.sync.dma_start(wih[0][:, k, :], d["wihT"][0, k * 128:(k + 1) * 128, :])
        for k in range(4):
            nc.sync.dma_start(whh[0][:, k, :], d["whhT"][0, k * 128:(k + 1) * 128, :])
        encT_sb = xpool.tile([128, 4, BL, S], F16, tag="encT")
        for k in range(4):
            nc.sync.dma_start(encT_sb[:, k, :, :], d["encT"][k * 128:(k + 1) * 128, :, :])
        encs_sb = xpool.tile([S, BL, H], F32R, tag="encs")
        nc.sync.dma_start(encs_sb[:], d["encs"][:])
        for k in range(4):
            nc.sync.dma_start(wih[1][:, k, :], d["wihT"][1, k * 128:(k + 1) * 128, :])
        for k in range(4):
            nc.sync.dma_start(whh[1][:, k, :], d["whhT"][1, k * 128:(k + 1) * 128, :])
        for l in range(L):
            for k2 in range(8):
                nc.sync.dma_start(lin_sb[l][:, k2, :], d["linT"][l, k2 * 128:(k2 + 1) * 128, :])

        catT = [xpool.tile([128, 4, Tp, BL], F16, tag=f"catT{l}", name=f"catT{l}")
                for l in range(L)]
        x2T_sb = xpool.tile([128, 4, NTp], F16, tag="x2T")
        x3T_sb = xpool.tile([128, 4, NTp], F16, tag="x3T")
        xin = [x1T_sb, x2T_sb]
        xout = [x2T_sb, x3T_sb]

        # ---- skewed two-layer pipeline ----
        pg_ctx = [tc.tile_pool(name=f"pg{l}", bufs=2, space="PSUM") for l in range(L)]
        pat_ctx = [tc.tile_pool(name=f"pat{l}", bufs=2, space="PSUM") for l in range(L)]
        p_g = [c.__enter__() for c in pg_ctx]
        p_at = [c.__enter__() for c in pat_ctx]

        g_groups = [{}, {}]
        ar_tiles = [{}, {}]
        E_tiles = [{}, {}]
        att_tiles = [{}, {}]
        c_prev = [None, None]

        def emit_xg_group(l, j):
            # bias + x@Wih.T for steps 4j..4j+3 (independent of h)
            g4 = p_g[l].tile([128, NGC, NSB, BL], F32, tag=f"g{l}", name=f"g{l}")
            g_groups[l][j] = g4
            t0 = j * NSB
            for gc in range(NGC):
                nc.tensor.matmul(g4[:, gc, :, :], bsum_sb[:, l, gc * 128:(gc + 1) * 128],
                                 ones[:, :NSB * BL], start=True, stop=False)
                for k in range(4):
                    nc.tensor.matmul(g4[:, gc, :, :],
                                     wih[l][:, k, gc * 128:(gc + 1) * 128],
                                     xin[l][:, k, t0 * BL:(t0 + NSB) * BL],
                                     start=False, stop=(k == 3), skip_group_check=True)

        def stage_whh(l, t):
            j, r = divmod(t, NSB)
            if t == 0:
                emit_xg_group(l, 0)
            if r == 2 and j + 1 <= (T - 1) // NSB:
                emit_xg_group(l, j + 1)
            if t > 0:
                g4 = g_groups[l][j]
                for gc in range(NGC):
                    for k in range(4):
                        nc.tensor.matmul(g4[:, gc, r, :],
                                         whh[l][:, k, gc * 128:(gc + 1) * 128],
                                         catT[l][:, k, t - 1, :],
                                         start=False, stop=(k == 3),
                                         skip_group_check=True)

        def stage_chain(l, t):
            # sigma chunks (host permute): i=0:4, f=4:8, o=8:12, g(x2)=12:16
            j, r = divmod(t, NSB)
            g4 = g_groups[l][j]
            if r == NSB - 1:
                g_groups[l].pop(j)
            gates = g4[:, :, r, :]
            sg = small.tile([128, NGC, BL], F16, tag=f"sg{l}", name=f"sg{l}")
            nc.scalar.activation(sg[:], gates, AF.Sigmoid)
            tg = small.tile([128, 4, BL], F16, tag=f"tg{l}", name=f"tg{l}")
            nc.vector.tensor_scalar(tg[:], sg[:, 12:16, :], 2.0, -1.0,
                                    op0=OP.mult, op1=OP.add)
            t2 = small.tile([128, 4, BL], F16, tag=f"t2{l}", name=f"t2{l}")
            nc.vector.tensor_tensor(t2[:], sg[:, 0:4, :], tg[:], op=OP.mult)
            c_new = cpool.tile([128, 4, BL], F16, tag=f"c{l}", name=f"c{l}")
            if t == 0:
                nc.vector.tensor_copy(c_new[:], t2[:])
            else:
                t1 = small.tile([128, 4, BL], F16, tag=f"t1{l}", name=f"t1{l}")
                nc.gpsimd.tensor_tensor(t1[:], sg[:, 4:8, :], c_prev[l][:], op=OP.mult)
                nc.vector.tensor_tensor(c_new[:], t1[:], t2[:], op=OP.add)
            c_prev[l] = c_new
            thc = small.tile([128, 4, BL], F16, tag=f"thc{l}", name=f"thc{l}")
            nc.scalar.activation(thc[:], c_new[:], AF.Tanh)
            nc.vector.tensor_tensor(catT[l][:, :, t, :], sg[:, 8:12, :], thc[:],
                                    op=OP.mult)

        def stage_scores(l, t):
            ar = p_at[l].tile([128, 128], F32, tag=f"at{l}", name=f"at{l}")
            ar_tiles[l][t] = ar
            nc.tensor.matmul(ar[0:S, A_SC:A_SC + BL], mask2_sb[:], ident8[:],
                             start=True, stop=False)
            for b in range(BL):
                for k in range(4):
                    nc.tensor.matmul(ar[0:S, A_SC + b:A_SC + b + 1],
                                     encT_sb[:, k, b, :],
                                     catT[l][:, k, t, b:b + 1],
                                     start=False, stop=(k == 3), skip_group_check=True)
            E = small.tile([S, BL], F32R, tag=f"E{l}", name=f"E{l}")
            E_tiles[l][t] = E
            nc.scalar.activation(E[:], ar[0:S, A_SC:A_SC + BL], AF.Exp)

        def stage_soft(l, t):
            ar = ar_tiles[l][t]
            E = E_tiles[l].pop(t)
            nc.tensor.matmul(ar[0:1, A_Z:A_Z + BL], ones60c[:], E[:],
                             start=True, stop=True)
            r_sb = small.tile([1, BL], F32R, tag=f"r{l}", name=f"r{l}")
            nc.vector.reciprocal(r_sb[:], ar[0:1, A_Z:A_Z + BL])
            nc.tensor.matmul(ar[:, A_RBC:A_RBC + BL], onesr[:], r_sb[:],
                             start=True, stop=True)
            rbc_sb = small.tile([128, BL], F32, tag=f"rb{l}", name=f"rb{l}")
            nc.vector.tensor_copy(rbc_sb[:], ar[:, A_RBC:A_RBC + BL])
            for b in range(BL):
                for k in range(4):
                    col = A_ATT + k * BL + b
                    nc.tensor.matmul(ar[:, col:col + 1],
                                     encs_sb[:, b, k * 128:(k + 1) * 128],
                                     E[:, b:b + 1], start=True, stop=True)
            att = small.tile([128, 4, BL], F16, tag=f"as{l}", name=f"as{l}")
            att_tiles[l][t] = att
            nc.vector.tensor_tensor(
                att[:], ar[:, A_ATT:A_ATT + 32].rearrange("p (a b) -> p a b", b=BL),
                rbc_sb[:].unsqueeze(1).broadcast_to((128, 4, BL)), op=OP.mult)

        def stage_lin(l, t):
            ar = ar_tiles[l].pop(t)
            att = att_tiles[l].pop(t)
            for m in range(4):
                cs = A_X2 + m * BL
                nc.tensor.matmul(ar[:, cs:cs + BL],
                                 linb_sb[:, l, m * 128:(m + 1) * 128], ones[:, :BL],
                                 start=True, stop=False)
                for k2 in range(8):
                    rhs = catT[l][:, k2, t, :] if k2 < 4 else att[:, k2 - 4, :]
                    nc.tensor.matmul(ar[:, cs:cs + BL],
                                     lin_sb[l][:, k2, m * 128:(m + 1) * 128], rhs,
                                     start=False, stop=(k2 == 7), skip_group_check=True)
            nc.vector.tensor_copy(
                xout[l][:, :, t * BL:(t + 1) * BL],
                ar[:, A_X2:A_X2 + 32].rearrange("p (a b) -> p a b", b=BL))

        for tick in range(T + D + 3):
            for l in range(L):
                t = tick - (0 if l == 0 else D)
                if 0 <= t < T:
                    stage_whh(l, t)
                if 0 <= t - 1 < T:
                    stage_scores(l, t - 1)
                if 0 <= t < T:
                    stage_chain(l, t)
                if 0 <= t - 2 < T:
                    stage_soft(l, t - 2)
                    stage_lin(l, t - 2)

        for c in reversed(pat_ctx):
            c.__exit__(None, None, None)
        for c in reversed(pg_ctx):
            c.__exit__(None, None, None)

        # ---- projection: out[pos, v] = x3T.T @ wT (fp16; host adds bias) ----
        with tc.tile_pool(name="pmm", bufs=2, space="PSUM") as p_mm:
            for vc in range(NVC):
                wv = wstr.tile([128, 4, VC], F16, tag="wv", name="wv")
                for k in range(4):
                    nc.sync.dma_start(wv[:, k, :],
                                      d["wT"][k * 128:(k + 1) * 128, vc * VC:(vc + 1) * VC])
                for m in range(4):
                    stg = ostg.tile([128, VC], F16, tag="st", name="st")
                    for n in range(VC // 512):
                        ps = p_mm.tile([128, 512], F32, tag="mm", name="mm")
                        for k in range(4):
                            nc.tensor.matmul(ps[:],
                                             x3T_sb[:, k, m * 128:(m + 1) * 128],
                                             wv[:, k, n * 512:(n + 1) * 512],
                                             start=(k == 0), stop=(k == 3))
                        eng = (nc.scalar.copy, nc.vector.tensor_copy)[(m * 2 + n) % 2]
                        eng(stg[:, n * 512:(n + 1) * 512], ps[:])
                    rows = 128 if m < 3 else NT - 384
                    nc.sync.dma_start(d["out"][m * 128:m * 128 + rows, vc * VC:(vc + 1) * VC],
                                      stg[:rows, :])


# ---------------------------------------------------------------------------
# host side
# ---------------------------------------------------------------------------
_CACHE = {}


def _get_modules():
    if "k" not in _CACHE:
        _CACHE["k"] = _build()
    return (_CACHE["k"],)


def _permute_gates(w):
    """[..., 4H] gate-major i,f,g,o -> [i, f, o, 2*g] (last axis)."""
    i, f, g, o = np.split(w, 4, axis=-1)
    return np.concatenate([i, f, o, 2.0 * g], axis=-1)


def _host_prep(inputs):
    f32, f16 = np.float32, np.float16
    indices = np.asarray(inputs["indices"]).astype(np.int64)
    emb = np.asarray(inputs["emb"], f32)
    enc = np.asarray(inputs["enc_output"], f32)
    de_lens = np.asarray(inputs["de_lens"]).astype(np.int64)
    w_ih = np.asarray(inputs["w_ih"], f32)
    w_hh = np.asarray(inputs["w_hh"], f32)
    bsum = np.asarray(inputs["b_ih"], f32) + np.asarray(inputs["b_hh"], f32)
    lin_w = np.asarray(inputs["lin_w"], f32)
    lin_b = np.asarray(inputs["lin_b"], f32)
    out_w = np.asarray(inputs["out_w"], f32)

    x1 = emb[indices]  # [B, T, H]
    mask = np.where(np.arange(S)[None, :] < de_lens[:, None], 0.0, NEG).astype(f32)

    wihT = _permute_gates(np.transpose(w_ih, (0, 2, 1))).astype(f16)
    whhT = _permute_gates(np.transpose(w_hh, (0, 2, 1))).astype(f16)
    bsum = _permute_gates(bsum).reshape(L, 1, G4).astype(f16)
    linT = np.ascontiguousarray(np.transpose(lin_w, (0, 2, 1))).astype(f16)
    linb = np.ascontiguousarray(lin_b.reshape(L, 1, H)).astype(f16)
    wT = np.ascontiguousarray(out_w.T).astype(f16)

    consts = {
        "wihT": np.ascontiguousarray(wihT), "whhT": np.ascontiguousarray(whhT),
        "bsum": bsum, "linT": linT, "linb": linb, "wT": wT,
        "ones": np.ones((1, H), f16),
        "ident8": np.eye(8, dtype=f16),
        "ones60c": np.ones((S, 1), f32),
        "onesr": np.ones((1, 128), f32),
    }
    in_maps = []
    for c in range(NCORES):
        bsl = slice(c * BL, (c + 1) * BL)
        x1c = x1[bsl]                      # [BL, T, H]
        encc = enc[bsl]                    # [BL, S, H]
        x1T = np.zeros((H, NTp), f16)
        x1T[:, :NT] = np.transpose(x1c, (2, 1, 0)).reshape(H, NT)
        in_maps.append({
            "x1T": x1T,
            "encT": np.ascontiguousarray(np.transpose(encc, (2, 0, 1))).astype(f16),
            "encs": np.ascontiguousarray(np.transpose(encc, (1, 0, 2))).astype(f32),
            "mask2": np.ascontiguousarray(mask[bsl]).astype(f16),
            **consts,
        })
    return in_maps


def _run_spmd(nc, in_maps):
    import time as _time
    from concourse.bass_utils import run_bass_kernel_spmd
    last = None
    for attempt in range(4):
        try:
            return run_bass_kernel_spmd(nc, in_maps, core_ids=list(range(NCORES)))
        except Exception as e:  # transient NRT_EXEC_UNIT_UNRECOVERABLE etc.
            last = e
            _time.sleep(2.0 * (attempt + 1))
    raise last


def kernel(**inputs) -> np.ndarray:
    (k,) = _get_modules()
    in_maps = _host_prep(inputs)
    res = _run_spmd(k, in_maps)
    out_b = np.asarray(inputs["out_b"], np.float32)
    logits = np.empty((B, T, V), np.float32)
    for c in range(NCORES):
        oc = res.results[c]["out"].astype(np.float32)      # [NT, V] t-major
        logits[c * BL:(c + 1) * BL] = oc.reshape(T, BL, V).transpose(1, 0, 2)
    logits += out_b
    return logits
